# revision 5
# baseline (speedup 1.0000x reference)
"""AGThenGemm: act = A @ W_prev (column-sharded), AllGather(act), out = act @ W_up (column-sharded).

Tensor-parallel across 8 NeuronCores:
  - W_prev sharded column-wise (D_local = D/8), W_up sharded column-wise (F_local = F/8).
  - A_prev replicated (pre-transposed on host so the contraction dim is on partitions).
  - GEMM1 emits act TRANSPOSED ([D_local, B]) so the AllGather's concat-along-first-axis
    concatenates along D, yielding act_T_global [D, B] directly usable as GEMM2's kxm.
  - Chunked over B so AllGather(chunk c) overlaps GEMM1(chunk c+1) on the TensorEngine.
"""

from contextlib import ExitStack

import numpy as np

import concourse.bass as bass
import concourse.tile as tile
from concourse import bacc, mybir
from concourse.bass_utils import run_bass_kernel_spmd
from concourse.kernels.tile_matmul import (
    K_TILE_OPTIONS,
    composable_matmul_tile_kernel,
    dma_from_dram_kxm,
    dma_from_dram_kxn,
    dma_to_dram_mxn,
    scalar_copyback,
)
from concourse._compat import max_divisible_size

N_CORES = 8
B, K_PREV, D, F = 2048, 4096, 4096, 16384
D_LOCAL = D // N_CORES
F_LOCAL = F // N_CORES
N_CHUNKS = 4


def _mm(tc, kxm_ap, kxn_ap, mxn_ap, max_k=2048, psum_n_bufs=2):
    """matmul_tile_kernel with coarser K tiles and double-buffered PSUM
    (the stock wrapper hardcodes psum_n_bufs=1, which serializes output
    blocks on PSUM eviction)."""
    tc.swap_default_side()
    k_dim = kxm_ap.shape[0]
    k_tile = max_divisible_size(
        k_dim, [s for s in [128, *K_TILE_OPTIONS] if s <= max_k]
    )
    num_bufs = k_dim // k_tile + 1
    with ExitStack() as ctx:
        kxm_pool = ctx.enter_context(tc.tile_pool(name="kxm_pool", bufs=num_bufs))
        kxn_pool = ctx.enter_context(tc.tile_pool(name="kxn_pool", bufs=num_bufs))
        kxm_producer, kxm_shape = dma_from_dram_kxm(kxm_pool, kxm_ap)
        kxn_producer, kxn_shape = dma_from_dram_kxn(kxn_pool, kxn_ap)
        mxn_consumer = dma_to_dram_mxn(mxn_ap)
        composable_matmul_tile_kernel(
            tc=tc,
            kxm_shape=kxm_shape,
            kxn_shape=kxn_shape,
            output_type=mxn_ap.dtype,
            kxm_producer=kxm_producer,
            kxn_producer=kxn_producer,
            mxn_subtile_reducer=scalar_copyback(),
            mxn_consumer=mxn_consumer,
            MAX_K_TILE_SIZE=max_k,
            psum_n_bufs=psum_n_bufs,
        )


def build_nc(
    b=B,
    k_prev=K_PREV,
    d_local=D_LOCAL,
    f_local=F_LOCAL,
    n_cores=N_CORES,
    n_chunks=N_CHUNKS,
    max_k=2048,
    psum_n_bufs=2,
    debug=False,
):
    nc = bacc.Bacc(
        "TRN2",
        target_bir_lowering=False,
        debug=debug,
        num_devices=n_cores,
    )
    dt = mybir.dt.float16
    d_global = d_local * n_cores

    a_t = nc.dram_tensor("a_t", [k_prev, b], dt, kind="ExternalInput")
    w_prev = nc.dram_tensor("w_prev", [k_prev, d_local], dt, kind="ExternalInput")
    w_up = nc.dram_tensor("w_up", [d_global, f_local], dt, kind="ExternalInput")
    out = nc.dram_tensor("out", [b, f_local], dt, kind="ExternalOutput")

    chunk = b // n_chunks
    groups = [list(range(n_cores))]

    with tile.TileContext(nc) as tc:
        with tc.tile_pool(name="dram", bufs=1, space="DRAM") as dram:
            ag_in = [
                dram.tile([d_local, chunk], dt, name=f"ag_in{c}")
                for c in range(n_chunks)
            ]
            ag_out = [
                dram.tile(
                    [d_global, chunk], dt, name=f"ag_out{c}", addr_space="Shared"
                )
                for c in range(n_chunks)
            ]
            for c in range(n_chunks):
                cs = slice(c * chunk, (c + 1) * chunk)
                # act_T[:, chunk c] = W_prev_shard^T @ A^T[:, chunk c]
                _mm(
                    tc,
                    w_prev.ap(),
                    a_t.ap()[:, cs],
                    ag_in[c][:],
                    max_k=max_k,
                    psum_n_bufs=psum_n_bufs,
                )
                nc.gpsimd.collective_compute(
                    "AllGather",
                    mybir.AluOpType.bypass,
                    replica_groups=groups,
                    ins=[ag_in[c].opt()],
                    outs=[ag_out[c].opt()],
                )
            for c in range(n_chunks):
                cs = slice(c * chunk, (c + 1) * chunk)
                # out[chunk c, :] = act_T_global[:, chunk c]^T @ W_up_shard
                _mm(
                    tc,
                    ag_out[c][:],
                    w_up.ap(),
                    out.ap()[cs, :],
                    max_k=max_k,
                    psum_n_bufs=psum_n_bufs,
                )
    nc.compile()
    return nc


_NC_CACHE = {}


def _get_nc():
    if "nc" not in _NC_CACHE:
        _NC_CACHE["nc"] = build_nc()
    return _NC_CACHE["nc"]


def run(A_prev, W_prev, W_up, **spmd_kwargs):
    A_t = np.ascontiguousarray(A_prev.T)
    in_maps = []
    for r in range(N_CORES):
        in_maps.append(
            {
                "a_t": A_t,
                "w_prev": np.ascontiguousarray(
                    W_prev[:, r * D_LOCAL : (r + 1) * D_LOCAL]
                ),
                "w_up": np.ascontiguousarray(W_up[:, r * F_LOCAL : (r + 1) * F_LOCAL]),
            }
        )
    nc = _get_nc()
    res = run_bass_kernel_spmd(
        nc, in_maps, core_ids=list(range(N_CORES)), **spmd_kwargs
    )
    out = np.concatenate([res.results[r]["out"] for r in range(N_CORES)], axis=1)
    return out, res


def kernel(A_prev, W_prev, W_up):
    return run(A_prev, W_prev, W_up)[0]


# revision 6
# speedup vs baseline: 1.2380x; 1.2380x over previous
"""AGThenGemm: act = A @ W_prev (column-sharded), AllGather(act), out = act @ W_up (column-sharded).

Tensor-parallel across 8 NeuronCores:
  - W_prev sharded column-wise (D_local = D/8), W_up sharded column-wise (F_local = F/8).
  - A_prev replicated (pre-transposed on host so the contraction dim is on partitions).
  - GEMM1 emits act TRANSPOSED ([D_local, B]) so the AllGather's concat-along-first-axis
    concatenates along D, yielding act_T_global [D, B] directly usable as GEMM2's kxm.
  - Chunked over B so AllGather(chunk c) overlaps GEMM1(chunk c+1) on the TensorEngine.

Custom GEMM core (vs stock matmul_tile_kernel): the inner loop rotates PSUM banks
between consecutive matmuls (k outer, m-subtile inner) so a matmul's PSUM drain never
serializes against the next matmul streaming into the same bank, and evicts PSUM via
the otherwise-idle VectorEngine. This is worth ~50 ns per matmul (measured 263 -> ~213).
"""

from contextlib import ExitStack

import numpy as np

import concourse.bass as bass
import concourse.tile as tile
from concourse import bacc, mybir
from concourse.bass_utils import run_bass_kernel_spmd

N_CORES = 8
B, K_PREV, D, F = 2048, 4096, 4096, 16384
D_LOCAL = D // N_CORES
F_LOCAL = F // N_CORES
N_CHUNKS = 4

P = 128
FREE = 512  # one PSUM bank of fp32


def _gemm(tc, pools, kxm_ap, kxn_ap, mxn_ap, kxm_cache=None, k_tile=512):
    """mxn += kxm^T @ kxn with PSUM bank rotation.

    kxm_ap: [K, M] DRAM view, kxn_ap: [K, N], mxn_ap: [M, N].
    kxm_cache: optional dict to reuse kxm SBUF tiles across calls with the
    same kxm_ap (keyed (id-tag, mt, kt)).
    """
    nc = tc.nc
    kxm_pool, kxn_pool, temps, psum = pools
    K, M = kxm_ap.shape
    N = kxn_ap.shape[1]
    assert K % k_tile == 0 and M % P == 0
    K_TILES = K // k_tile
    K_SUB = k_tile // P
    M_TILE = min(512, M)
    M_SUB = M_TILE // P
    M_TILES = M // M_TILE
    N_TILE = min(FREE, N)
    N_TILES = N // N_TILE

    kxm3 = kxm_ap.rearrange("(ko p) m -> p ko m", p=P)  # [P, K/P, M]
    kxn3 = kxn_ap.rearrange("(ko p) n -> p ko n", p=P)
    mxn3 = mxn_ap.rearrange("(mo p) n -> p mo n", p=P)

    dt_in = kxm_ap.dtype
    for mt in range(M_TILES):
        for nt in range(N_TILES):
            ps_tiles = [
                psum.tile([P, N_TILE], mybir.dt.float32, name=f"ps{mi}", tag=f"ps{mi}")
                for mi in range(M_SUB)
            ]
            for kt in range(K_TILES):
                key = (mt, kt)
                if kxm_cache is not None and key in kxm_cache:
                    kxm_t = kxm_cache[key]
                else:
                    kxm_t = kxm_pool.tile(
                        [P, K_SUB, M_TILE], dt_in, name="kxm_t", tag="kxm_t"
                    )
                    nc.sync.dma_start(
                        kxm_t[:],
                        kxm3[:, kt * K_SUB : (kt + 1) * K_SUB,
                             mt * M_TILE : (mt + 1) * M_TILE],
                    )
                    if kxm_cache is not None:
                        kxm_cache[key] = kxm_t
                kxn_t = kxn_pool.tile(
                    [P, K_SUB, N_TILE], dt_in, name="kxn_t", tag="kxn_t"
                )
                nc.sync.dma_start(
                    kxn_t[:],
                    kxn3[:, kt * K_SUB : (kt + 1) * K_SUB,
                         nt * N_TILE : (nt + 1) * N_TILE],
                )
                for ki in range(K_SUB):
                    for mi in range(M_SUB):
                        nc.tensor.matmul(
                            ps_tiles[mi][:],
                            kxm_t[:, ki, mi * P : (mi + 1) * P],
                            kxn_t[:, ki, :],
                            start=(kt == 0 and ki == 0),
                            stop=(kt == K_TILES - 1 and ki == K_SUB - 1),
                        )
            out_t = temps.tile([P, M_SUB, N_TILE], mxn_ap.dtype, name="out_t", tag="out_t")
            for mi in range(M_SUB):
                nc.vector.tensor_copy(out_t[:, mi, :], ps_tiles[mi][:])
            nc.sync.dma_start(
                mxn3[:, mt * M_SUB : (mt + 1) * M_SUB,
                     nt * N_TILE : (nt + 1) * N_TILE],
                out_t[:],
            )


def build_nc(
    b=B,
    k_prev=K_PREV,
    d_local=D_LOCAL,
    f_local=F_LOCAL,
    n_cores=N_CORES,
    n_chunks=N_CHUNKS,
    k_tile=512,
    debug=False,
):
    nc = bacc.Bacc(
        "TRN2",
        target_bir_lowering=False,
        debug=debug,
        num_devices=n_cores,
    )
    dt = mybir.dt.float16
    d_global = d_local * n_cores

    a_t = nc.dram_tensor("a_t", [k_prev, b], dt, kind="ExternalInput")
    w_prev = nc.dram_tensor("w_prev", [k_prev, d_local], dt, kind="ExternalInput")
    w_up = nc.dram_tensor("w_up", [d_global, f_local], dt, kind="ExternalInput")
    out = nc.dram_tensor("out", [b, f_local], dt, kind="ExternalOutput")

    chunk = b // n_chunks
    groups = [list(range(n_cores))]
    g1_k_tiles = k_prev // k_tile
    g2_k_tiles = d_global // k_tile

    with tile.TileContext(nc) as tc:
        with ExitStack() as ctx:
            dram = ctx.enter_context(tc.tile_pool(name="dram", bufs=1, space="DRAM"))
            # w_prev tiles cached across all GEMM1 chunk calls
            g1_kxm = ctx.enter_context(
                tc.tile_pool(name="g1_kxm", bufs=g1_k_tiles * (d_local // 512 or 1))
            )
            g1_kxn = ctx.enter_context(tc.tile_pool(name="g1_kxn", bufs=4))
            g2_kxm = ctx.enter_context(tc.tile_pool(name="g2_kxm", bufs=g2_k_tiles + 1))
            g2_kxn = ctx.enter_context(tc.tile_pool(name="g2_kxn", bufs=4))
            temps = ctx.enter_context(tc.tile_pool(name="temps", bufs=3))
            psum = ctx.enter_context(tc.tile_pool(name="psum", bufs=2, space="PSUM"))

            ag_in = [
                dram.tile([d_local, chunk], dt, name=f"ag_in{c}")
                for c in range(n_chunks)
            ]
            ag_out = [
                dram.tile(
                    [d_global, chunk], dt, name=f"ag_out{c}", addr_space="Shared"
                )
                for c in range(n_chunks)
            ]

            g1_pools = (g1_kxm, g1_kxn, temps, psum)
            g2_pools = (g2_kxm, g2_kxn, temps, psum)
            w_prev_cache = {}
            for c in range(n_chunks):
                cs = slice(c * chunk, (c + 1) * chunk)
                # act_T[:, chunk c] = W_prev_shard^T @ A^T[:, chunk c]
                _gemm(
                    tc,
                    g1_pools,
                    w_prev.ap(),
                    a_t.ap()[:, cs],
                    ag_in[c][:],
                    kxm_cache=w_prev_cache,
                    k_tile=k_tile,
                )
                nc.gpsimd.collective_compute(
                    "AllGather",
                    mybir.AluOpType.bypass,
                    replica_groups=groups,
                    ins=[ag_in[c].opt()],
                    outs=[ag_out[c].opt()],
                )
            for c in range(n_chunks):
                cs = slice(c * chunk, (c + 1) * chunk)
                # out[chunk c, :] = act_T_global[:, chunk c]^T @ W_up_shard
                _gemm(
                    tc,
                    g2_pools,
                    ag_out[c][:],
                    w_up.ap(),
                    out.ap()[cs, :],
                    kxm_cache={},
                    k_tile=k_tile,
                )
    nc.compile()
    return nc


_NC_CACHE = {}


def _get_nc():
    if "nc" not in _NC_CACHE:
        _NC_CACHE["nc"] = build_nc()
    return _NC_CACHE["nc"]


def run(A_prev, W_prev, W_up, **spmd_kwargs):
    A_t = np.ascontiguousarray(A_prev.T)
    in_maps = []
    for r in range(N_CORES):
        in_maps.append(
            {
                "a_t": A_t,
                "w_prev": np.ascontiguousarray(
                    W_prev[:, r * D_LOCAL : (r + 1) * D_LOCAL]
                ),
                "w_up": np.ascontiguousarray(W_up[:, r * F_LOCAL : (r + 1) * F_LOCAL]),
            }
        )
    nc = _get_nc()
    res = run_bass_kernel_spmd(
        nc, in_maps, core_ids=list(range(N_CORES)), **spmd_kwargs
    )
    out = np.concatenate([res.results[r]["out"] for r in range(N_CORES)], axis=1)
    return out, res


def kernel(A_prev, W_prev, W_up):
    return run(A_prev, W_prev, W_up)[0]
